# revision 60
# baseline (speedup 1.0000x reference)
"""Chamfer-KL loss kernel for Trainium2 (Bass/Tile).

Math: KL(N_i || N_j) summed over d for all pairs reduces to a rank-10
inner product.  With a = preds, b = gts, d = 4:

  KL[i,j] = 0.5 * (F_i . G_j)
  F_i = [exp(la_i)+mu_a_i^2 (4), -2*mu_a_i (4), 1, -sum_d la_i]
  G_j = [exp(-lb_j) (4), mu_b_j*exp(-lb_j) (4),
         sum_d mu_b_j^2*exp(-lb_j) + sum_d lb_j - 4, 1]

The kernel computes M = (-F).G = -2*KL so that both chamfer reductions
become MAXes, and

  out = -0.5 * (sum_j max_i M[i,j]  +  sum_i max_j M[i,j])

Sharding: data-parallel over batch, one batch element per NeuronCore
(bs=8 over 8 cores).  Per core the 2048x2048 matrix M is produced
tile-by-tile by the TensorEngine (fp16 matmuls, rank 10) into PSUM and
never hits HBM; maxes are reduced flash-style on the fly:
  - ScalarE copies each PSUM tile to SBUF as fp16 (the only engine that
    can both read PSUM and cast cheaply; 0.83ns/elem)
  - VectorE does per-tile row-max with ONE tensor_tensor_scan over the
    two tile halves (state = max(lo[t], state, hi[t]); 1024 steps) and
    keeps a running elementwise column-max (fp16 TT, 2x mode).  Both lag
    one tile behind the copy so DVE never stalls on ScalarE.
  - column maxes cross partitions at the end via two parallel paths:
    half through 8 fp16 PE transposes + one DVE free-axis reduce, half
    through gpsimd partition_all_reduce(max) + ScalarE copy+accum (with
    the -0.5 output scale folded into the accum / the sum matmul).
(GpSimd/Pool fails walrus' engine check for any tensor_tensor/reduce,
and DMA-compute supports only 'add', so DVE is the only elementwise
min/max engine; the scan replaces the old 4-op fold chain at half the
DVE cost, taking the steady-state from DVE 2569ns/tile to 2254ns/tile
against ScalarE's 1892ns/tile copy.)
"""

import numpy as np

import concourse.bacc as bacc
import concourse.bass as bass
import concourse.mybir as mybir
import concourse.tile as tile
from concourse.masks import make_identity

BS = 8          # batch size == number of cores
N = 2048        # points per cloud
D = 4           # point dimension
P = 128         # SBUF partitions
PT = N // P     # 16 points per partition in the raw layout
K = 2 * D + 2   # 10 live feature dims
NBLK = 512      # moving-operand columns per matmul (one PSUM bank fp32)
NB = N // NBLK  # 4 j-blocks per i-block
G = N // P      # 16 i-blocks
H = N // 2      # scan half-width

F32 = mybir.dt.float32
F16 = mybir.dt.float16
AX = mybir.AxisListType.X
OP = mybir.AluOpType
ACTF = mybir.ActivationFunctionType

NEG_INF = -1.0e30

# Schedule-tuning knobs (the tile scheduler is a global list scheduler,
# so locally-reasonable choices interact; these were swept empirically).
CFG = {
    "ft0_eng": "vector",    # engine for tile 0's lhsT copy
    "ftar_eng": "scalar",   # engine for the ft_ar copy: scalar|vector
    "ftb_eng": "vector",    # engine for the ft_b copy: scalar|vector
    "chunk_order": (0, 2, 3, 1),
    "pg_skip": True,
    "dma": "D",             # input-DMA queue assignment variant
    "fin": "pool2",         # finalize: pool2 (gpsimd half) | dve (all DVE)
    "nblk": 512,            # moving-operand columns per matmul
    "pool_lo": True,        # gpsimd finalize takes the low column half
    "copy0_split": 0,
    "sbufs": 4,
    "warm": 12,
    # Manual schedule hint: park the F-side feature ops at ~4.7us so the
    # list scheduler keeps the schedule-critical G chain in front of
    # them (their DMA inputs land last anyway).
    "f_wait": 0.0047,
    # Tile 1's copy: VectorE pre-copies the head columns in its idle
    # window (pg1 is ready ~1.1us before ScalarE finishes copy0).
    "copy1_split": 650,
}


def _chamfer_tile_kernel(tc, out_dram, mu_a, la, mu_b, lb):
    nc = tc.nc

    sing = tc.alloc_tile_pool(name="sing", bufs=1)
    work = tc.alloc_tile_pool(name="work", bufs=1)
    s_pool = tc.alloc_tile_pool(name="s_pool", bufs=CFG.get("sbufs", 4))

    # Identity first: gpsimd is otherwise idle and the PE pre-warm
    # depends on it.  fp16-only (all transposes are fp16 now).  High
    # priority so the mu_b SWDGE descriptor-gen (also on Pool) cannot
    # preempt it and stall the PE warm-up.
    ident16 = sing.tile([P, P], F16)
    with tc.high_priority():
        make_identity(nc, ident16)

    # ---- load raw inputs: [2048, 4] -> [128, 16, 4] (row chunks) ----
    # All four on HWDGE queues (the SWDGE alternative stalls the Pool
    # engine's make_identity, which gates the PE warm-up).  G-side
    # inputs (lb, mu_b) first: the G side gates the first matmuls.
    t_ma = work.tile([P, PT, D], F32)
    t_la = work.tile([P, PT, D], F32)
    t_mb = work.tile([P, PT, D], F32)
    t_lb = work.tile([P, PT, D], F32)
    dma_cfgs = {
        "A": (nc.gpsimd, nc.sync, nc.scalar, nc.sync),
        "B": (nc.sync, nc.scalar, nc.sync, nc.scalar),
        "C": (nc.scalar, nc.sync, nc.sync, nc.scalar),
        "D": (nc.gpsimd, nc.sync, nc.sync, nc.scalar),
    }
    for (t, src), eng in zip(
            ((t_mb, mu_b), (t_lb, lb), (t_la, la), (t_ma, mu_a)),
            dma_cfgs[CFG["dma"]]):
        eng.dma_start(out=t, in_=src.rearrange("(p t) d -> p t d", p=P))

    # ---- PE pre-warm ----
    # The HAM clock gate keeps a cold PE at half rate for its first
    # ~3.4us; burn no-dep junk matmuls so the feature transposes and the
    # first real matmuls run at full clock.
    with tc.tile_pool(name="warm_psum", bufs=1, space="PSUM") as warm_psum:
        junk = warm_psum.tile([P, P], F32, tag="warm")
        for _ in range(CFG.get("warm", 12)):
            nc.tensor.matmul(junk, ident16, ident16, start=True, stop=True)

    # ---- feature matrices in interleaved layout [128, 16, 10], fp16 ----
    # g128[p, t, k] = feature k of gt point (16*p + t); f128 holds the
    # NEGATED pred features so the matmul yields -2*KL.
    f128 = work.tile([P, PT, K], F16)
    g128 = work.tile([P, PT, K], F16)

    # G side first (high priority: its transposes + copies gate the
    # first matmuls, and the readiness-based scheduler would otherwise
    # run the earlier-ready F-side ops first).
    with tc.high_priority():
        nc.scalar.activation(out=g128[:, :, 0:D], in_=t_lb, func=ACTF.Exp,
                             scale=-1.0)
        nc.vector.tensor_mul(g128[:, :, D:2 * D], t_mb, g128[:, :, 0:D])
        t_q2 = work.tile([P, PT, D], F32)
        nc.vector.tensor_mul(t_q2, t_mb, g128[:, :, D:2 * D])
        t_r = work.tile([P, PT], F32)
        nc.vector.tensor_reduce(t_r, t_q2, axis=AX, op=OP.add)
        t_slb = work.tile([P, PT], F32)
        nc.vector.tensor_reduce(t_slb, t_lb, axis=AX, op=OP.add)
        # g128 k=8: (sum_d mub^2 ivb - 4) + sum_d lb, in one fused op
        nc.vector.scalar_tensor_tensor(
            out=g128[:, :, 2 * D], in0=t_r, scalar=-float(D), in1=t_slb,
            op0=OP.add, op1=OP.add)
        nc.vector.memset(g128[:, :, 2 * D + 1], 1.0)

    # F side (negated): f0:3 = -(exp(la) + mu_a^2), f4:7 = +2*mu_a,
    # f8 = -1, f9 = +sum_d la.  Optional manual wait hint pushes these
    # behind the schedule-critical G chain in the static scheduler.
    import contextlib
    fw = CFG.get("f_wait", 0.0)
    with (tc.tile_wait_until(fw) if fw else contextlib.nullcontext()):
        t_sq = work.tile([P, PT, D], F32)
        nc.vector.tensor_mul(t_sq, t_ma, t_ma)
        t_e = work.tile([P, PT, D], F16)
        nc.scalar.activation(out=t_e, in_=t_la, func=ACTF.Exp)
        nc.vector.scalar_tensor_tensor(
            out=f128[:, :, 0:D], in0=t_sq, scalar=-1.0, in1=t_e,
            op0=OP.mult, op1=OP.subtract)
        nc.vector.tensor_scalar_mul(f128[:, :, D:2 * D], t_ma, 2.0)
        nc.vector.memset(f128[:, :, 2 * D], -1.0)
        with nc.allow_low_precision(reason="sum of 4 logvars fits fp16"):
            nc.vector.tensor_reduce(
                f128[:, :, 2 * D + 1], t_la, axis=AX, op=OP.add)

    f128f = f128.rearrange("p t k -> p (t k)")
    g128f = g128.rearrange("p t k -> p (t k)")

    # ---- transpose features so k lands on partitions ----
    # Both sides become [10, 2048] fp16 (k on partitions 0..10, points on
    # the free axis).  G halves copy on ScalarE, F halves on VectorE
    # (fp16 PSUM reads get DVE 2x_1p: 658ns vs ACT 1038ns), interleaved
    # with the transposes so copies overlap later transposes.
    with tc.tile_pool(name="pro_psum", bufs=1, space="PSUM") as pro_psum:
        p_gt_a = pro_psum.tile([K, H], F16, tag="gta")
        p_gt_b = pro_psum.tile([K, H], F16, tag="gtb")
        p_ft_a = pro_psum.tile([K, H], F16, tag="fta")
        p_ft_b = pro_psum.tile([K, H], F16, tag="ftb")
        gt_a = work.tile([K, H], F16)
        gt_b = work.tile([K, H], F16)
        # ft_a is split so tile 0's stationary operand (its first 128
        # cols) lands in SBUF as early as possible.
        ft_0 = work.tile([K, P], F16)
        ft_ar = work.tile([K, H - P], F16)
        ft_b = work.tile([K, H], F16)

        def tr_batch(dst, srcf, lo):
            for h in range(lo, lo + 8):
                nc.tensor.transpose(
                    dst[:, P * (h % 8):P * (h % 8 + 1)],
                    srcf[:, K * h:K * (h + 1)], ident16)

        # Matmul-gating copies (gt halves + tile 0's lhsT piece) on
        # VectorE; the rest on ScalarE so the coalesced DVE semaphore the
        # first Ldweights wait on doesn't extend past gt_b.
        _eng = {"scalar": nc.scalar.copy, "vector": nc.vector.tensor_copy}
        with tc.high_priority():
            tr_batch(p_gt_a, g128f, 0)
            nc.vector.tensor_copy(gt_a, p_gt_a)
        tr_batch(p_ft_a, f128f, 0)
        with tc.high_priority(offset=50):
            _eng[CFG.get("ft0_eng", "vector")](ft_0, p_ft_a[:, 0:P])
        tr_batch(p_gt_b, g128f, 8)
        nc.vector.tensor_copy(gt_b, p_gt_b)
        import contextlib as _cl
        arw = CFG.get("ftar_wait", 0.0)
        with (tc.tile_wait_until(arw) if arw else _cl.nullcontext()):
            _eng[CFG["ftar_eng"]](ft_ar, p_ft_a[:, P:H])
        tr_batch(p_ft_b, f128f, 8)
        bw = CFG.get("ftb_wait", 0.0)
        with (tc.tile_wait_until(bw) if bw else _cl.nullcontext()):
            _eng[CFG["ftb_eng"]](ft_b, p_ft_b)

    # ---- main loop: rank-10 matmuls + flash-style max reductions ----
    # scratch[:, g, :] holds tile g's row-max scan; its last column is
    # the final row-max for that tile's 128 rows.
    scratch = sing.tile([P, G, H], F16)
    cm = sing.tile([P, N], F16)          # running column-max
    import concourse.bass_isa as bass_isa

    # Column-slice offload: the otherwise-idle Pool engine takes the
    # LAST `W` columns of every tile's column-max via per-tile
    # partition_all_reduce(max); row 0 of each result is extracted to
    # `collect` by DMA (also idle) and max-combined across tiles by one
    # final 16-channel all-reduce.  This narrows VectorE's running
    # column-max TT from 2048 to CMW columns, moving the steady state
    # from DVE-bound (2254ns/tile) toward ScalarE's 1892ns/tile copy.
    W = CFG.get("pool_slice", 768)
    CMW = N - W
    if W:
        collect = sing.tile([G - 1, W], F16)
        pr_last = [None]

    def slice_reduce(g, sg):
        # per-tile column max of sg[:, CMW:] -> collect[g] (via DMA; an
        # engine copy can't write at a partition offset).  Tile 15 keeps
        # its full all-reduce result; the finalize merges its row 0
        # directly, skipping the ~2.2us DMA fixed path on the tail.
        prb = s_pool.tile([P, W], F16, tag="pr", bufs=2, name="prb")
        nc.gpsimd.partition_all_reduce(
            prb, sg[:, CMW:N], channels=P,
            reduce_op=bass_isa.ReduceOp.max)
        if g < G - 1:
            nc.sync.dma_start(out=collect[g:g + 1, :], in_=prb[0:1, :])
        else:
            pr_last[0] = prb

    def rm_scan(g, sg):
        nc.vector.tensor_tensor_scan(
            scratch[:, g, :], sg[:, 0:H], sg[:, H:N],
            initial=NEG_INF, op0=OP.max, op1=OP.max)

    with tc.tile_pool(name="mm_psum", bufs=2, space="PSUM") as mm_psum:
        # Dummy rotation: the pool's first slot reuses the PSUM banks of
        # the (still-draining) feature transposes, which would make tile
        # 0's matmuls wait for all four feature copies.  Burn one slot
        # so tile 0 lands on the untouched bank pair instead.
        if CFG["pg_skip"]:
            pg_skip = mm_psum.tile([P, N], F32, tag="mm", name="pg_skip")
        sg_prev = None
        for g in range(G):
            pg = mm_psum.tile([P, N], F32, tag="mm")
            if g == 0:
                lhsT = ft_0
            elif g < 8:
                lhsT = ft_ar[:, P * (g - 1):P * g]
            else:
                lhsT = ft_b[:, P * (g % 8):P * (g % 8 + 1)]
            for n in range(NB):
                rhs_t = gt_a if n < 2 else gt_b
                nc.tensor.matmul(
                    pg[:, NBLK * n:NBLK * (n + 1)],
                    lhsT,
                    rhs_t[:, NBLK * (n % 2):NBLK * (n % 2 + 1)],
                    start=True, stop=True)
            # Tile 0's fp16 copy lands directly in cm (its "update" is
            # the initialization), later tiles go to the rotating pool.
            sg = cm if g == 0 else s_pool.tile([P, N], F16, tag="s")
            csp = CFG.get("copy0_split", 0)
            if g == 0 and csp:
                # DVE is idle waiting for this very copy: let it take
                # the first csp columns (PSUM reads are 1x on DVE, so
                # csp is chosen to equalize both engines' finish time).
                nc.vector.tensor_copy(sg[:, 0:csp], pg[:, 0:csp])
                nc.scalar.copy(sg[:, csp:N], pg[:, csp:N])
            elif g == 1 and CFG.get("copy1_dve", False):
                # Tile 1's copy on otherwise-idle VectorE (1x from PSUM,
                # but it unserializes ScalarE's copy0+copy1 fill chain).
                nc.vector.tensor_copy(sg, pg)
            elif g == 1 and CFG.get("copy1_split", 0):
                # pg1 is ready ~1.1us before ScalarE finishes copy0; DVE
                # is idle until then, so it pre-copies the head columns.
                c1s = CFG["copy1_split"]
                nc.vector.tensor_copy(sg[:, 0:c1s], pg[:, 0:c1s])
                nc.scalar.copy(sg[:, c1s:N], pg[:, c1s:N])
            elif g == G - 1 and W and CFG.get("copy15_split", False):
                # Last tile: DVE copies the Pool-slice columns in
                # parallel with ScalarE so tile 15's all-reduce (the
                # head of the serial finalize chain) starts one
                # ScalarE-copy earlier.
                nc.vector.tensor_copy(sg[:, CMW:N], pg[:, CMW:N])
                nc.scalar.copy(sg[:, 0:CMW], pg[:, 0:CMW])
            else:
                nc.scalar.copy(sg, pg)
            # Row-max scan + column-max both lag one iteration so they
            # consume the previous, already-copied sg -- no DVE stall on
            # ScalarE.  (Tile 0 lives in cm: no column TT update, and the
            # Pool slice-reduce reads its columns out of cm directly.)
            if sg_prev is not None:
                rm_scan(g - 1, sg_prev)
                if W:
                    slice_reduce(g - 1, sg_prev)
                if g >= 2:
                    nc.vector.tensor_tensor(
                        cm[:, 0:CMW], cm[:, 0:CMW], sg_prev[:, 0:CMW],
                        OP.max)
            sg_prev = sg
        # Epilogue: last tile un-lagged.  Column-max in two chunks so
        # the finalize transposes start per-chunk; the row-max scan runs
        # last (it only gates the tiny row-sum at the very end).
        if W:
            # The 15-channel all-reduce over `collect` is issued FIRST:
            # its inputs (tiles 0..14) are complete one tile earlier
            # than tile 15's own all-reduce, so Pool pipelines the two.
            prfin = sing.tile([G - 1, W], F16)
            nc.gpsimd.partition_all_reduce(
                prfin, collect, channels=G - 1,
                reduce_op=bass_isa.ReduceOp.max)
            slice_reduce(G - 1, sg_prev)
        hw_c = CMW // 2
        for lo, hi in ((0, hw_c), (hw_c, CMW)):
            nc.vector.tensor_tensor(
                cm[:, lo:hi], cm[:, lo:hi], sg_prev[:, lo:hi], OP.max)
        rm_scan(G - 1, sg_prev)
        if CFG.get("warm2", 0):
            # Late PE re-warm: a fresh pool slot (rotates onto the bank
            # pair free since tile 14's copy) + a wait hint park these
            # junk matmuls just before the finalize transposes, so the
            # p-state model doesn't drop PE to half rate after its long
            # idle.
            jt = mm_psum.tile([P, N], F32, tag="mm", name="jt")
            with tc.tile_wait_until(CFG.get("warm2_wait", 0.044)):
                for _ in range(CFG["warm2"]):
                    nc.tensor.matmul(jt[:, 0:P], ident16, ident16,
                                     start=True, stop=True)

    # ---- finalize ----
    # The W-column slice was reduced per-tile by Pool during the loop;
    # one final 16-channel all-reduce over `collect` + a ScalarE
    # copy+accum (scale=-0.5) turns it into a scalar.  The remaining CMW
    # cm columns cross partitions via fp16 PE transposes + two DVE
    # free-axis reduces (split so the first starts after 8 transposes).
    with tc.tile_pool(name="fin_psum", bufs=1, space="PSUM") as fin_psum:
        ngrp = CMW // P
        # colmax: ngrp transpose-path column-max groups plus the row-max
        # sum folded in as the last column, so one reduce totals them.
        colmax = sing.tile([P, ngrp + 1], F32)
        nc.vector.tensor_reduce(
            colmax[:, ngrp:ngrp + 1], scratch[:, :, H - 1], axis=AX,
            op=OP.add)

        rowmax = sing.tile([1, W], F16)
        nc.vector.tensor_tensor(
            rowmax, prfin[0:1, :], pr_last[0][0:1, :], OP.max)
        csink = sing.tile([1, W], F32)
        acc = sing.tile([1, 1], F32)
        nc.scalar.activation(out=csink, in_=rowmax,
                             func=ACTF.Copy, scale=-0.5, accum_out=acc)

        fin_a = fin_psum.tile([P, CMW], F16, tag="fina")
        for t in range(ngrp):
            nc.tensor.transpose(
                fin_a[:, P * t:P * (t + 1)],
                cm[:, P * t:P * (t + 1)], ident16)
        nsplit = min(8, ngrp)
        nc.vector.tensor_reduce(
            colmax[:, 0:nsplit],
            fin_a[:, 0:P * nsplit].rearrange("p (t c) -> p t c", c=P),
            axis=AX, op=OP.max)
        if ngrp > nsplit:
            nc.vector.tensor_reduce(
                colmax[:, nsplit:ngrp],
                fin_a[:, P * nsplit:].rearrange("p (t c) -> p t c", c=P),
                axis=AX, op=OP.max)

        stot = sing.tile([P, 1], F32)
        nc.vector.tensor_reduce(stot, colmax, axis=AX, op=OP.add)
        # ones = -0.5 so the cross-partition sum matmul applies the
        # final scale for free.
        ones = sing.tile([P, 1], F32)
        nc.vector.memset(ones, -0.5)

        tot = fin_psum.tile([1, 1], F32, tag="tot")
        nc.tensor.matmul(tot, stot, ones, start=True, stop=True)
        res = sing.tile([1, 1], F32)
        nc.vector.tensor_tensor(res, acc, tot, OP.add)
        nc.sync.dma_start(out=out_dram, in_=res)

    s_pool.release()
    work.release()
    sing.release()


def build_nc():
    nc = bacc.Bacc(trn_type="TRN2", target_bir_lowering=False, debug=False)
    mu_a = nc.dram_tensor("mu_a", [N, D], F32, kind="ExternalInput").ap()
    la_ = nc.dram_tensor("la", [N, D], F32, kind="ExternalInput").ap()
    mu_b = nc.dram_tensor("mu_b", [N, D], F32, kind="ExternalInput").ap()
    lb_ = nc.dram_tensor("lb", [N, D], F32, kind="ExternalInput").ap()
    out = nc.dram_tensor("out", [1, 1], F32, kind="ExternalOutput").ap()
    with tile.TileContext(nc) as tc:
        _chamfer_tile_kernel(tc, out, mu_a, la_, mu_b, lb_)
    nc.compile()
    return nc


_NC_CACHE = None


def _get_nc():
    global _NC_CACHE
    if _NC_CACHE is None:
        _NC_CACHE = build_nc()
    return _NC_CACHE


def _in_maps(mu_preds, logvar_preds, mu_gts, logvar_gts):
    maps = []
    for c in range(BS):
        maps.append({
            "mu_a": np.ascontiguousarray(mu_preds[c], dtype=np.float32),
            "la": np.ascontiguousarray(logvar_preds[c], dtype=np.float32),
            "mu_b": np.ascontiguousarray(mu_gts[c], dtype=np.float32),
            "lb": np.ascontiguousarray(logvar_gts[c], dtype=np.float32),
        })
    return maps


def run(mu_preds, logvar_preds, mu_gts, logvar_gts, trace=False):
    """Returns (out [8] float32, exec_time_ns or None)."""
    from concourse.bass_utils import run_bass_kernel_spmd
    nc = _get_nc()
    maps = _in_maps(mu_preds, logvar_preds, mu_gts, logvar_gts)
    r = run_bass_kernel_spmd(nc, maps, core_ids=list(range(BS)), trace=trace)
    out = np.array([r.results[c]["out"][0, 0] for c in range(BS)],
                   dtype=np.float32)
    return out, r.exec_time_ns


def kernel(mu_preds, logvar_preds, mu_gts, logvar_gts):
    out, _ = run(mu_preds, logvar_preds, mu_gts, logvar_gts, trace=False)
    return out
